# revision 9
# baseline (speedup 1.0000x reference)
"""Trainium2 Bass kernel for nn_Classifier (segment_reduce).

Computation (reference):
    local  = relu(x @ W1.T)            # [T, 50] @ [50, 400] -> [T, 400]
    feat   = mean over windows of J=24 # [T//24, 400]
    logits = feat @ W2.T               # [T//24, 400] @ [400, 10]

Strategy: pure data parallel over 8 NeuronCores (x sharded along T).
Per core (T_c = 98304 rows = 4096 windows), per supergroup G (3072 xp
cols = 6144 rows = 256 windows, 128 per shard-half):
  - Host packs the x shard TRANSPOSED + bf16 into xp [128, 49152]:
    rows 0-49 hold x_shard[:49152].T, rows 64-113 hold x_shard[49152:].T.
    The contraction dim (n=50) sits on partitions so matmul1 needs no
    on-device transpose; the two halves row-tile the PE array
    (tile_position (0,0)/(64,0)) for 2x concurrent matmuls.
  - matmul1: stationary = xp 128-col tile, moving = W1.T [50, 400]
    -> psum pair [128t, 2x512] fp32.
  - relu evacuation psum->sbuf bf16 alternates ScalarE / VectorE —
    the throughput-limiting stage (PSUM-sourced ops run 1x on both).
  - pooling on the PE: strip boundaries align with every 6th tile
    (768 rows = 32 windows exactly), so 6 shared 0/1 stationaries
    produce feat in NATURAL window order across 4 col-strips.
  - feat -> sbuf (ScalarE), k-transposed via ONE xbar DMA transpose
    [128, 1024] -> [128, 8, 128] (no PE transposes, no perm scramble),
    then matmul2 accumulates logits over 4 k-chunks per half.
  - The tail is lagged one supergroup so the PE never stalls on it.
"""

import sys

sys.path.insert(0, "/opt/trn_rl_repo")

import numpy as np
import ml_dtypes

import bass_rust
import concourse.bass as bass
import concourse.mybir as mybir
import concourse.tile as tile
from concourse.bass_utils import run_bass_kernel_spmd
from concourse.tile import TileContext
from concourse.vector_clock import ScopedClock

# ---------------------------------------------------------------------------
# Wait-count legalization (monkeypatch).
#
# This walrus build accepts at most 1 sync-wait per instruction (2 for
# EventSemaphore), but Tile's scheduler and tail drain can attach more,
# failing codegen with "Too many sync wait commands". Spread excess waits
# onto same-engine NOPs inserted immediately before the instruction.
# ---------------------------------------------------------------------------

_orig_add = TileContext._add_instruction


def _wait_cap(inst):
    return 2 if type(inst).__name__ == "InstEventSemaphore" else 1


def _patched_add_instruction(self, inst):
    si = inst.sync_info
    cap = _wait_cap(inst)
    if (
        si is not None
        and si.on_wait
        and len(si.on_wait) > cap
        and inst.engine != mybir.EngineType.Unassigned
    ):
        waits = list(si.on_wait)
        for w in waits[:-cap]:
            nop = bass_rust.InstNoOp(
                name=f"I-waitfix-{self.nc.next_id()}",
                opcode="NoOp",
                engine=inst.engine,
                ins=[],
                outs=[],
            )
            nop.sync_info = mybir.SyncInfo(on_wait=[w], on_update=[])
            _orig_add(self, nop)
        inst.sync_info = mybir.SyncInfo(
            on_wait=waits[-cap:], on_update=list(si.on_update or [])
        )
    _orig_add(self, inst)


def _patched_drain_and_barrier(self, tick_clock, wait_clock):
    nc = self.nc
    drain_inst = nc.sync.drain()
    wait_clock.add_sem_waits(
        drain_inst.ins, ScopedClock({None: tick_clock.global_clock})
    )
    mi = drain_inst.ins
    si = mi.sync_info
    waits = list(si.on_wait) if (si and si.on_wait) else []
    if len(waits) > 1:
        mi.sync_info = mybir.SyncInfo(
            on_wait=[waits[-1]], on_update=list(si.on_update or [])
        )
        for w in waits[:-1]:
            nop = nc.sync.nop()
            nop.ins.sync_info = mybir.SyncInfo(on_wait=[w], on_update=[])

    nc.all_engine_barrier()
    assert self.sems is not None
    popped = nc._tile_sem_poison_stack.pop()
    assert popped is self._sem_poison
    nc.clear_and_free_semaphores(list(self.sems.allocated().values()))
    nc.all_engine_barrier()


TileContext._add_instruction = _patched_add_instruction
TileContext._drain_and_barrier = _patched_drain_and_barrier

# ---------------------------------------------------------------------------
# Problem constants (hardcoded per the harness contract)
# ---------------------------------------------------------------------------

J = 24
T, N, K, C = 786432, 50, 400, 10
NCORES = 8
TC = T // NCORES          # 98304 rows per core
H = TC // 2               # 49152 cols per half in xp
B_CORE = TC // J          # 4096 windows per core
NG = 16                   # supergroup iterations per repeat
CHUNK = 24 * 128          # 3072 xp columns per supergroup
NP = 24                   # mm1 pairs (128-col x tiles) per supergroup

BF16 = mybir.dt.bfloat16
F32 = mybir.dt.float32
nbf = ml_dtypes.bfloat16


def _build_pmats():
    """Six pooling stationaries P_j [128, 32] packed as [128, 192].
    Strip boundaries align with every 6th 128-row tile (768 rows = 32
    windows), so P_j depends only on j = tile_index % 6:
    P_j[tau, (128*j + tau) // 24] = 1."""
    pm = np.zeros((128, 192), np.float32)
    for j_ in range(6):
        for tau in range(128):
            pm[tau, 32 * j_ + (128 * j_ + tau) // 24] = 1.0
    return pm.astype(nbf)


def _build_w2tp(W2):
    """W2/24 arranged for matmul2 over 128-row k-chunks: w2tp[r, 10c+cc] =
    W2[cc, 128c+r]/24 (zero past k=400)."""
    w = np.zeros((128, 40), np.float32)
    for c in range(4):
        k0 = 128 * c
        kn = min(400, k0 + 128) - k0
        w[:kn, 10 * c : 10 * c + 10] = (W2.astype(np.float32).T[k0 : k0 + kn]) / J
    return w.astype(nbf)


def _build_nc(repeat: int = 1):
    """repeat>1 re-runs the whole computation in one NEFF — used by the
    test harness to measure device time differentially."""
    nc = bass.Bass()
    xp_d = nc.declare_dram_parameter("xp", [128, H], BF16, isOutput=False)
    w1t_d = nc.declare_dram_parameter("w1t", [50, 400], BF16, isOutput=False)
    w2tp_d = nc.declare_dram_parameter("w2tp", [128, 40], BF16, isOutput=False)
    pm_d = nc.declare_dram_parameter("pmats", [128, 192], BF16, isOutput=False)
    out_d = nc.declare_dram_parameter("logits", [B_CORE, 10], F32, isOutput=True)

    act = mybir.ActivationFunctionType

    with TileContext(nc) as tc:
        with (
            tc.tile_pool(name="consts", bufs=1) as cpool,
            tc.tile_pool(name="xchunks", bufs=3) as xpool,
            tc.tile_pool(name="relu", bufs=26) as rpool,
            tc.tile_pool(name="featsb", bufs=2) as fspool,
            tc.tile_pool(name="featT", bufs=2) as ftpool,
            tc.tile_pool(name="lsb", bufs=2) as lpool,
            tc.tile_pool(name="mm1ps", bufs=2, space="PSUM") as mm1pool,
            tc.tile_pool(name="featps", bufs=3, space="PSUM") as featpool,
            tc.tile_pool(name="logps", bufs=1, space="PSUM") as logpool,
        ):
            # W1T staged at partition offsets 0 and 64 — the moving operand
            # must share the stationary's base partition (array row offset).
            w1t = cpool.tile([128, 400], BF16)
            w2tp = cpool.tile([128, 40], BF16)
            pmats = cpool.tile([128, 192], BF16)
            nc.sync.dma_start(out=w1t[0:50, :], in_=w1t_d[:])
            nc.sync.dma_start(out=w1t[64:114, :], in_=w1t_d[:])
            nc.sync.dma_start(out=w2tp[:], in_=w2tp_d[:])
            nc.sync.dma_start(out=pmats[:], in_=pm_d[:])

            def emit_tail_a(st):
                """feat psum -> sbuf (+pad memset) + xbar transpose."""
                feat_sb = fspool.tile([128, 2, 512], BF16, name="fsb")
                nc.gpsimd.memset(feat_sb[:, :, 400:512], 0.0)
                for hh in range(2):
                    nc.scalar.activation(
                        feat_sb[:, hh, 0:400], st["featps"][hh][:, 0:400], act.Relu
                    )
                featT = ftpool.tile([128, 8, 128], BF16, name="ftT")
                nc.sync.dma_start_transpose(
                    out=featT[:], in_=feat_sb[:].rearrange("p two k -> p (two k)")
                )
                st["featT"] = featT

            def emit_tail_b(st):
                """matmul2 over k-chunks + logits psum -> sbuf -> DRAM."""
                featT = st["featT"]
                lps = logpool.tile([128, 32], F32, name="lps")
                for hh in range(2):
                    for c_ in range(4):
                        nc.tensor.matmul(
                            lps[:, 16 * hh : 16 * hh + 10],
                            featT[:, 4 * hh + c_, :],
                            w2tp[:, 10 * c_ : 10 * c_ + 10],
                            start=(c_ == 0),
                            stop=(c_ == 3),
                        )
                lsb = lpool.tile([128, 2, 10], F32, name="lsb")
                src = lps[:].rearrange("p (two k) -> p two k", two=2)[:, :, 0:10]
                nc.vector.tensor_copy(out=lsb[:], in_=src)
                g = st["G"]
                dst = out_d[:].rearrange(
                    "(two w) c -> w two c", two=2
                )[128 * g : 128 * g + 128]
                nc.sync.dma_start(out=dst, in_=lsb[:])

            prev = None  # supergroup awaiting tail-a (feat evac + transpose)
            prev2 = None  # supergroup awaiting tail-b (mm2 + store)
            for G in [g for _ in range(repeat) for g in range(NG)]:
                xc = xpool.tile([128, CHUNK], BF16, name="xc")
                nc.sync.dma_start(
                    out=xc[:], in_=xp_d[:, G * CHUNK : (G + 1) * CHUNK]
                )

                if prev is not None:
                    emit_tail_a(prev)

                # ---- Phase A: matmul1 + relu evacuation (24 pairs) ----
                pairs = []
                for i in range(NP):
                    tcol = i * 128
                    ps = mm1pool.tile([128, 1024], F32, name="ps")
                    for hh in range(2):
                        rb = 64 * hh
                        nc.tensor.matmul(
                            ps[:, 512 * hh : 512 * hh + 400],
                            xc[rb : rb + 50, tcol : tcol + 128],
                            w1t[rb : rb + 50, :],
                            start=True,
                            stop=True,
                        )
                    rl = rpool.tile([128, 2, 400], BF16, name="rl", bufs=26)
                    src = ps[:, :].rearrange("p (two k) -> p two k", two=2)[
                        :, :, 0:400
                    ]
                    # measured sustained: ACT 650ns vs DVE 1040ns per pair
                    # (FD=800, PSUM-sourced) -> 14:10 split
                    if i % 12 in (0, 1, 3, 5, 7, 8, 10):
                        nc.scalar.activation(rl[:], src, act.Relu)
                    else:
                        nc.vector.tensor_scalar_max(rl[:], src, 0.0)
                    pairs.append(rl)

                # ---- Phase B: pooling matmuls (natural window order) ----
                featps = [
                    featpool.tile([128, 512], F32, name="featps") for _ in range(2)
                ]
                for i in range(NP):
                    s, j_ = i // 6, i % 6
                    rl = pairs[i]
                    for hh in range(2):
                        nc.tensor.matmul(
                            featps[hh][32 * s : 32 * s + 32, 0:400],
                            pmats[:, 32 * j_ : 32 * j_ + 32],
                            rl[:, hh, :],
                            start=(j_ == 0),
                            stop=(j_ == 5),
                            tile_position=(0, 32 * s),
                        )

                if prev2 is not None:
                    emit_tail_b(prev2)
                prev2 = prev
                prev = {"G": G, "featps": featps}

            # drain the tail pipeline
            if prev is not None:
                emit_tail_a(prev)
            if prev2 is not None:
                emit_tail_b(prev2)
            emit_tail_b(prev)
    return nc


_NC = {}


def _get_nc(repeat: int = 1):
    if repeat not in _NC:
        _NC[repeat] = _build_nc(repeat)
    return _NC[repeat]


def prepare_in_maps(x: np.ndarray, W1: np.ndarray, W2: np.ndarray):
    assert x.shape == (T, N) and W1.shape == (K, N) and W2.shape == (C, K)

    w1t = np.ascontiguousarray(W1.T.astype(nbf))          # [50, 400]
    w2tp = _build_w2tp(W2)                                 # [128, 40]
    pmats = _build_pmats()

    xb = x.astype(nbf)
    in_maps = []
    for c in range(NCORES):
        shard = xb[c * TC : (c + 1) * TC]                  # [98304, 50]
        xp = np.zeros((128, H), nbf)
        xp[0:50] = shard[0:H].T
        xp[64:114] = shard[H:].T
        in_maps.append(
            {
                "xp": xp,
                "w1t": w1t,
                "w2tp": w2tp,
                "pmats": pmats,
            }
        )
    return in_maps


def kernel(x: np.ndarray, W1: np.ndarray, W2: np.ndarray) -> np.ndarray:
    in_maps = prepare_in_maps(x, W1, W2)
    nc = _get_nc()
    res = run_bass_kernel_spmd(nc, in_maps, core_ids=list(range(NCORES)))
    out = np.concatenate(
        [res.results[c]["logits"] for c in range(NCORES)], axis=0
    )
    return out.astype(np.float32)


# revision 11
# speedup vs baseline: 1.1401x; 1.1401x over previous
"""Trainium2 Bass kernel for nn_Classifier (segment_reduce).

Computation (reference):
    local  = relu(x @ W1.T)            # [T, 50] @ [50, 400] -> [T, 400]
    feat   = mean over windows of J=24 # [T//24, 400]
    logits = feat @ W2.T               # [T//24, 400] @ [400, 10]

Strategy: pure data parallel over 8 NeuronCores (x sharded along T).
Per core (T_c = 98304 rows = 4096 windows), per supergroup G (3072 xp
cols = 6144 rows = 256 windows, 128 per shard-half):
  - Host packs the x shard TRANSPOSED + bf16 into xp [128, 49152]:
    rows 0-49 hold x_shard[:49152].T, rows 64-113 hold x_shard[49152:].T.
    The contraction dim (n=50) sits on partitions so matmul1 needs no
    on-device transpose; the two halves row-tile the PE array
    (tile_position (0,0)/(64,0)) for 2x concurrent matmuls.
  - matmul1: stationary = xp 128-col tile, moving = W1.T [50, 400]
    -> psum pair [128t, 2x512] fp32.
  - relu evacuation psum->sbuf bf16 alternates ScalarE / VectorE —
    the throughput-limiting stage (PSUM-sourced ops run 1x on both).
  - pooling on the PE: strip boundaries align with every 6th tile
    (768 rows = 32 windows exactly), so 6 shared 0/1 stationaries
    produce feat in NATURAL window order across 4 col-strips.
  - feat -> sbuf (ScalarE), k-transposed via ONE xbar DMA transpose
    [128, 1024] -> [128, 8, 128] (no PE transposes, no perm scramble),
    then matmul2 accumulates logits over 4 k-chunks per half.
  - The tail is lagged one supergroup so the PE never stalls on it.
"""

import sys

sys.path.insert(0, "/opt/trn_rl_repo")

import numpy as np
import ml_dtypes

import bass_rust
import concourse.bass as bass
import concourse.mybir as mybir
import concourse.tile as tile
from concourse.bass_utils import run_bass_kernel_spmd
from concourse.tile import TileContext
from concourse.vector_clock import ScopedClock

# ---------------------------------------------------------------------------
# Wait-count legalization (monkeypatch).
#
# This walrus build accepts at most 1 sync-wait per instruction (2 for
# EventSemaphore), but Tile's scheduler and tail drain can attach more,
# failing codegen with "Too many sync wait commands". Spread excess waits
# onto same-engine NOPs inserted immediately before the instruction.
# ---------------------------------------------------------------------------

_orig_add = TileContext._add_instruction


def _wait_cap(inst):
    return 2 if type(inst).__name__ == "InstEventSemaphore" else 1


def _patched_add_instruction(self, inst):
    si = inst.sync_info
    cap = _wait_cap(inst)
    if (
        si is not None
        and si.on_wait
        and len(si.on_wait) > cap
        and inst.engine != mybir.EngineType.Unassigned
    ):
        waits = list(si.on_wait)
        for w in waits[:-cap]:
            nop = bass_rust.InstNoOp(
                name=f"I-waitfix-{self.nc.next_id()}",
                opcode="NoOp",
                engine=inst.engine,
                ins=[],
                outs=[],
            )
            nop.sync_info = mybir.SyncInfo(on_wait=[w], on_update=[])
            _orig_add(self, nop)
        inst.sync_info = mybir.SyncInfo(
            on_wait=waits[-cap:], on_update=list(si.on_update or [])
        )
    _orig_add(self, inst)


def _patched_drain_and_barrier(self, tick_clock, wait_clock):
    nc = self.nc
    drain_inst = nc.sync.drain()
    wait_clock.add_sem_waits(
        drain_inst.ins, ScopedClock({None: tick_clock.global_clock})
    )
    mi = drain_inst.ins
    si = mi.sync_info
    waits = list(si.on_wait) if (si and si.on_wait) else []
    if len(waits) > 1:
        mi.sync_info = mybir.SyncInfo(
            on_wait=[waits[-1]], on_update=list(si.on_update or [])
        )
        for w in waits[:-1]:
            nop = nc.sync.nop()
            nop.ins.sync_info = mybir.SyncInfo(on_wait=[w], on_update=[])

    nc.all_engine_barrier()
    assert self.sems is not None
    popped = nc._tile_sem_poison_stack.pop()
    assert popped is self._sem_poison
    nc.clear_and_free_semaphores(list(self.sems.allocated().values()))
    nc.all_engine_barrier()


TileContext._add_instruction = _patched_add_instruction
TileContext._drain_and_barrier = _patched_drain_and_barrier

# ---------------------------------------------------------------------------
# Problem constants (hardcoded per the harness contract)
# ---------------------------------------------------------------------------

J = 24
T, N, K, C = 786432, 50, 400, 10
NCORES = 8
TC = T // NCORES          # 98304 rows per core
H = TC // 2               # 49152 cols per half in xp
B_CORE = TC // J          # 4096 windows per core
NG = 16                   # supergroup iterations per repeat
CHUNK = 24 * 128          # 3072 xp columns per supergroup
NP = 24                   # mm1 pairs (128-col x tiles) per supergroup

BF16 = mybir.dt.bfloat16
F32 = mybir.dt.float32
nbf = ml_dtypes.bfloat16


def _build_pmats():
    """Six pooling stationaries P_j [128, 32] packed as [128, 192].
    Strip boundaries align with every 6th 128-row tile (768 rows = 32
    windows), so P_j depends only on j = tile_index % 6:
    P_j[tau, (128*j + tau) // 24] = 1."""
    pm = np.zeros((128, 192), np.float32)
    for j_ in range(6):
        for tau in range(128):
            pm[tau, 32 * j_ + (128 * j_ + tau) // 24] = 1.0
    return pm.astype(nbf)


def _build_w2tp(W2):
    """W2/24 arranged for matmul2 over 128-row k-chunks: w2tp[r, 10c+cc] =
    W2[cc, 128c+r]/24 (zero past k=400)."""
    w = np.zeros((128, 40), np.float32)
    for c in range(4):
        k0 = 128 * c
        kn = min(400, k0 + 128) - k0
        w[:kn, 10 * c : 10 * c + 10] = (W2.astype(np.float32).T[k0 : k0 + kn]) / J
    return w.astype(nbf)


def _build_nc(repeat: int = 1):
    """repeat>1 re-runs the whole computation in one NEFF — used by the
    test harness to measure device time differentially."""
    nc = bass.Bass()
    xp_d = nc.declare_dram_parameter("xp", [128, H], BF16, isOutput=False)
    w1t_d = nc.declare_dram_parameter("w1t", [50, 400], BF16, isOutput=False)
    w2tp_d = nc.declare_dram_parameter("w2tp", [128, 40], BF16, isOutput=False)
    pm_d = nc.declare_dram_parameter("pmats", [128, 192], BF16, isOutput=False)
    out_d = nc.declare_dram_parameter("logits", [B_CORE, 10], F32, isOutput=True)

    act = mybir.ActivationFunctionType

    with TileContext(nc) as tc:
        with (
            tc.tile_pool(name="consts", bufs=1) as cpool,
            tc.tile_pool(name="xchunks", bufs=3) as xpool,
            tc.tile_pool(name="relu", bufs=26) as rpool,
            tc.tile_pool(name="featsb", bufs=2) as fspool,
            tc.tile_pool(name="featT", bufs=2) as ftpool,
            tc.tile_pool(name="lsb", bufs=2) as lpool,
            tc.tile_pool(name="mm1ps", bufs=4, space="PSUM") as mm1pool,
            tc.tile_pool(name="featps", bufs=3, space="PSUM") as featpool,
            tc.tile_pool(name="logps", bufs=1, space="PSUM") as logpool,
        ):
            # W1T staged at partition offsets 0 and 64 — the moving operand
            # must share the stationary's base partition (array row offset).
            w1t = cpool.tile([128, 400], BF16)
            w2tp = cpool.tile([128, 40], BF16)
            pmats = cpool.tile([128, 192], BF16)
            nc.sync.dma_start(out=w1t[0:50, :], in_=w1t_d[:])
            nc.sync.dma_start(out=w1t[64:114, :], in_=w1t_d[:])
            nc.sync.dma_start(out=w2tp[:], in_=w2tp_d[:])
            nc.sync.dma_start(out=pmats[:], in_=pm_d[:])

            def emit_tail_a(st):
                """feat psum -> sbuf (+pad memset) + xbar transpose."""
                feat_sb = fspool.tile([128, 2, 512], BF16, name="fsb")
                nc.gpsimd.memset(feat_sb[:, :, 400:512], 0.0)
                for hh in range(2):
                    nc.scalar.activation(
                        feat_sb[:, hh, 0:400], st["featps"][hh][:, 0:400], act.Relu
                    )
                featT = ftpool.tile([128, 8, 128], BF16, name="ftT")
                nc.sync.dma_start_transpose(
                    out=featT[:], in_=feat_sb[:].rearrange("p two k -> p (two k)")
                )
                st["featT"] = featT

            def emit_tail_b(st):
                """matmul2 over k-chunks + logits psum -> sbuf -> DRAM."""
                featT = st["featT"]
                lps = logpool.tile([128, 32], F32, name="lps")
                for hh in range(2):
                    for c_ in range(4):
                        nc.tensor.matmul(
                            lps[:, 16 * hh : 16 * hh + 10],
                            featT[:, 4 * hh + c_, :],
                            w2tp[:, 10 * c_ : 10 * c_ + 10],
                            start=(c_ == 0),
                            stop=(c_ == 3),
                        )
                lsb = lpool.tile([128, 2, 10], F32, name="lsb")
                src = lps[:].rearrange("p (two k) -> p two k", two=2)[:, :, 0:10]
                nc.vector.tensor_copy(out=lsb[:], in_=src)
                g = st["G"]
                dst = out_d[:].rearrange(
                    "(two w) c -> w two c", two=2
                )[128 * g : 128 * g + 128]
                nc.sync.dma_start(out=dst, in_=lsb[:])

            prev = None  # supergroup awaiting tail-a (feat evac + transpose)
            prev2 = None  # supergroup awaiting tail-b (mm2 + store)
            for G in [g for _ in range(repeat) for g in range(NG)]:
                xc = xpool.tile([128, CHUNK], BF16, name="xc")
                nc.sync.dma_start(
                    out=xc[:], in_=xp_d[:, G * CHUNK : (G + 1) * CHUNK]
                )

                if prev is not None:
                    emit_tail_a(prev)

                # ---- Phase A: matmul1 + relu evacuation (48 halves) ----
                # Per-half [128, 512] psum tiles (one bank, bufs=4) keep the
                # evac engines fed without the bufs=2 ping-pong idle.
                # measured sustained: ACT 650ns vs DVE 1040ns per FD=800
                # (PSUM-sourced) -> 29:19 split over the 48 FD=400 halves.
                halves = []
                for i in range(NP):
                    tcol = i * 128
                    for hh in range(2):
                        rb = 64 * hh
                        psh = mm1pool.tile([128, 512], F32, name="psh")
                        nc.tensor.matmul(
                            psh[:, 0:400],
                            xc[rb : rb + 50, tcol : tcol + 128],
                            w1t[rb : rb + 50, :],
                            start=True,
                            stop=True,
                        )
                        rlh = rpool.tile([128, 400], BF16, name="rl", bufs=52)
                        if (2 * i + hh) % 5 in (0, 2, 4):
                            nc.scalar.activation(rlh[:], psh[:, 0:400], act.Relu)
                        else:
                            nc.vector.tensor_scalar_max(
                                rlh[:], psh[:, 0:400], 0.0
                            )
                        halves.append(rlh)

                # ---- Phase B: pooling matmuls (natural window order) ----
                featps = [
                    featpool.tile([128, 512], F32, name="featps") for _ in range(2)
                ]
                for i in range(NP):
                    s, j_ = i // 6, i % 6
                    for hh in range(2):
                        nc.tensor.matmul(
                            featps[hh][32 * s : 32 * s + 32, 0:400],
                            pmats[:, 32 * j_ : 32 * j_ + 32],
                            halves[2 * i + hh][:, 0:400],
                            start=(j_ == 0),
                            stop=(j_ == 5),
                            tile_position=(0, 32 * s),
                        )

                if prev2 is not None:
                    emit_tail_b(prev2)
                prev2 = prev
                prev = {"G": G, "featps": featps}

            # drain the tail pipeline
            if prev is not None:
                emit_tail_a(prev)
            if prev2 is not None:
                emit_tail_b(prev2)
            emit_tail_b(prev)
    return nc


_NC = {}


def _get_nc(repeat: int = 1):
    if repeat not in _NC:
        _NC[repeat] = _build_nc(repeat)
    return _NC[repeat]


def prepare_in_maps(x: np.ndarray, W1: np.ndarray, W2: np.ndarray):
    assert x.shape == (T, N) and W1.shape == (K, N) and W2.shape == (C, K)

    w1t = np.ascontiguousarray(W1.T.astype(nbf))          # [50, 400]
    w2tp = _build_w2tp(W2)                                 # [128, 40]
    pmats = _build_pmats()

    xb = x.astype(nbf)
    in_maps = []
    for c in range(NCORES):
        shard = xb[c * TC : (c + 1) * TC]                  # [98304, 50]
        xp = np.zeros((128, H), nbf)
        xp[0:50] = shard[0:H].T
        xp[64:114] = shard[H:].T
        in_maps.append(
            {
                "xp": xp,
                "w1t": w1t,
                "w2tp": w2tp,
                "pmats": pmats,
            }
        )
    return in_maps


def kernel(x: np.ndarray, W1: np.ndarray, W2: np.ndarray) -> np.ndarray:
    in_maps = prepare_in_maps(x, W1, W2)
    nc = _get_nc()
    res = run_bass_kernel_spmd(nc, in_maps, core_ids=list(range(NCORES)))
    out = np.concatenate(
        [res.results[c]["logits"] for c in range(NCORES)], axis=0
    )
    return out.astype(np.float32)
